# revision 5
# baseline (speedup 1.0000x reference)
"""Per-patch dynamic conv (nn_DynaMicConv) as a Bass/Tile kernel on 8 TRN2 cores.

Math: for each patch p of a 14x14 grid over a 224x224 image, out[b, :, p] =
W[p] @ patch_pixels[b, p] + bias[p], i.e. 196 independent [64,768] x [768,768]
matmuls. This is DMA-bound: the weight stack is 462 MB and every byte is read
once.

Sharding: patch-parallel. P=196 patches are padded to 200 and split 25 per
core; each core reads only its weight slice, its patch pixels, and writes its
[25, 64, 768] output slice.

Layouts are precomputed on host so every device DMA is a large fully
contiguous transfer:
  w    [25, 128, 6*768]  per patch: partition k holds W[p, o, kc*128+k] for
                         kc-major, o-minor -> matmul rhs chunks [128, 768]
  xp   [128, 25*6*64]    partition k holds patch pixels for (patch, kchunk,
                         batch) -> matmul lhsT (stationary) chunks [128, 64]
  bias [25, 768]
  out  [25, 64, 768]

Compute per patch: PSUM[64, 768] = sum_kc lhsT_kc.T @ rhs_kc (+ bias via a
ones[1,64] stationary matmul with start=True). Matmul dtype selects the
traffic/accuracy point (MODE): f16 halves DMA bytes vs f32r at ~2x its error.
PSUM -> SBUF copy on DVE, store on ACT's HWDGE ring so the SP ring streams
weights uninterrupted.
"""

import numpy as np

import concourse.bacc as bacc
import concourse.mybir as mybir
import concourse.tile as tile
from concourse.bass_utils import run_bass_kernel_spmd

B, CIN, IMG, PS, G = 64, 3, 224, 16, 14
P = G * G                 # 196 patches
COUT = 768
K = CIN * PS * PS         # 768 contraction
KCH = K // 128            # 6 k-chunks
NCORES = 8
PPC = (P + NCORES - 1) // NCORES   # 25 patches per core (padded)
PPAD = PPC * NCORES                # 200

F32 = mybir.dt.float32

# matmul input dtype: 'f16' (half DMA traffic, ~3e-4 rel err),
# 'f32r' (full fp32 traffic, ~1.5e-4), 'bf16' (half traffic, ~2e-3)
MODE = "f16"
_DTYPES = {
    "f32r": (mybir.dt.float32r, np.float32),
    "f16": (mybir.dt.float16, np.float16),
    "bf16": (mybir.dt.bfloat16, None),  # np dtype resolved lazily via ml_dtypes
}

# store outputs as fp16 (halves store traffic; adds ~1.4e-4 rms rounding)
OUT_F16 = True

# test.py hooks: set TRACE=True before calling kernel() to profile; the
# BassKernelResults of the last run lands in LAST_RESULT.
TRACE = False
TRACE_CORES = [0]
LAST_RESULT = None

_CACHE = {}


def _np_dtype(mode):
    mdt, ndt = _DTYPES[mode]
    if ndt is None:
        import ml_dtypes
        ndt = ml_dtypes.bfloat16
    return mdt, ndt


def _build(mode):
    mdt, _ = _np_dtype(mode)
    odt = mybir.dt.float16 if OUT_F16 else F32
    nc = bacc.Bacc("TRN2", target_bir_lowering=False, debug=False)
    w_d = nc.dram_tensor("w", [PPC, 128, KCH * COUT], mdt, kind="ExternalInput")
    x_d = nc.dram_tensor("xp", [PPC, 128, KCH * B], mdt, kind="ExternalInput")
    b_d = nc.dram_tensor("bias", [PPC, COUT], mdt, kind="ExternalInput")
    ones_d = nc.dram_tensor("ones", [1, B], mdt, kind="ExternalInput")
    o_d = nc.dram_tensor("out", [PPC, B, COUT], odt, kind="ExternalOutput")

    with tile.TileContext(nc) as tc:
        with (
            tc.tile_pool(name="const", bufs=1) as cpool,
            tc.tile_pool(name="wp", bufs=8) as wpool,
            tc.tile_pool(name="bp", bufs=8) as bpool,
            tc.tile_pool(name="xp", bufs=8) as xpool,
            tc.tile_pool(name="op", bufs=6) as opool,
            tc.tile_pool(name="ps", bufs=3, space="PSUM") as pspool,
        ):
            ones = cpool.tile([1, B], mdt)
            nc.scalar.dma_start(ones[:], ones_d[:])

            for p in range(PPC):
                wt = wpool.tile([128, KCH * COUT], mdt, tag="w")
                nc.sync.dma_start(wt[:], w_d[p])
                bt = bpool.tile([1, COUT], mdt, tag="b")
                nc.scalar.dma_start(bt[:], b_d[p])
                xt = xpool.tile([128, KCH * B], mdt, tag="x")
                nc.scalar.dma_start(xt[:], x_d[p])

                ps1 = pspool.tile([B, 512], F32, tag="ps1")
                ps2 = pspool.tile([B, 256], F32, tag="ps2")
                nc.tensor.matmul(ps1[:], ones[:], bt[:, 0:512],
                                 start=True, stop=False)
                nc.tensor.matmul(ps2[:], ones[:], bt[:, 512:768],
                                 start=True, stop=False)
                for kc in range(KCH):
                    lhs = xt[:, kc * B: (kc + 1) * B]
                    last = kc == KCH - 1
                    nc.tensor.matmul(ps1[:], lhs,
                                     wt[:, kc * COUT: kc * COUT + 512],
                                     start=False, stop=last)
                    nc.tensor.matmul(ps2[:], lhs,
                                     wt[:, kc * COUT + 512: (kc + 1) * COUT],
                                     start=False, stop=last)

                ot = opool.tile([B, COUT], odt, tag="o")
                nc.vector.tensor_copy(ot[:, 0:512], ps1[:])
                nc.vector.tensor_copy(ot[:, 512:768], ps2[:])
                nc.scalar.dma_start(o_d[p], ot[:])
    nc.compile()
    return nc


def _prep(x, W, b, mode):
    _, ndt = _np_dtype(mode)
    # patch pixels, k-transposed: xp[p, k, b] with k = c*256 + r*16 + s
    xp = (x.reshape(B, CIN, G, PS, G, PS)
           .transpose(2, 4, 1, 3, 5, 0)
           .reshape(P, K, B))
    # -> [P, 128(kpart), KCH, B]
    xr = np.zeros((PPAD, 128, KCH, B), dtype=ndt)
    xr[:P] = xp.reshape(P, KCH, 128, B).transpose(0, 2, 1, 3).astype(ndt)

    # weights: w[p, kpart, kc*COUT + o] = W[p, o, kc*128 + kpart]
    wr = np.zeros((PPAD, 128, KCH * COUT), dtype=ndt)
    wr[:P] = (W.reshape(P, COUT, KCH, 128)
               .transpose(0, 3, 2, 1)
               .reshape(P, 128, KCH * COUT).astype(ndt))

    br = np.zeros((PPAD, COUT), dtype=ndt)
    br[:P] = b.astype(ndt)
    onesv = np.ones((1, B), dtype=ndt)

    in_maps = []
    for c in range(NCORES):
        sl = slice(c * PPC, (c + 1) * PPC)
        in_maps.append({
            "w": np.ascontiguousarray(wr[sl]),
            "xp": np.ascontiguousarray(xr[sl].reshape(PPC, 128, KCH * B)),
            "bias": np.ascontiguousarray(br[sl]),
            "ones": onesv,
        })
    return in_maps


def _post(results):
    out = np.empty((P, B, COUT), dtype=np.float32)
    for c in range(NCORES):
        lo = c * PPC
        hi = min((c + 1) * PPC, P)
        out[lo:hi] = results[c]["out"][: hi - lo].astype(np.float32)
    # [P, B, COUT] -> [B, COUT, G, G]
    return np.ascontiguousarray(out.transpose(1, 2, 0)).reshape(B, COUT, G, G)


def kernel(x, W, b):
    global LAST_RESULT
    x = np.ascontiguousarray(np.asarray(x, dtype=np.float32))
    W = np.ascontiguousarray(np.asarray(W, dtype=np.float32))
    b = np.ascontiguousarray(np.asarray(b, dtype=np.float32))
    in_maps = _prep(x, W, b, MODE)
    key = ("nc", MODE)
    if key not in _CACHE:
        _CACHE[key] = _build(MODE)
    res = run_bass_kernel_spmd(
        _CACHE[key], in_maps, core_ids=list(range(NCORES)),
        trace=TRACE, trace_cores=TRACE_CORES,
    )
    LAST_RESULT = res
    return _post(res.results)
